# revision 11
# baseline (speedup 1.0000x reference)
"""GCNConv (N=10000, E=640000, D=128) on 8 Trainium2 NeuronCores.

Math: out = diag(dis) (A + I) diag(dis) x W + bias, dis = deg^-1/2 with deg
over edge_index[0] (+1 self-loop).  Since the edge weight factorizes as
dis[row]*dis[col], fold dis[row] into a host-prescaled table
g = diag(dis) x  and dis[col] into a post-scale.  The aggregation then
becomes a sum of DENSE block matmuls against an integer-count adjacency:

    aggT[d, c] = sum_j  g_j[s, d]^T  @  A_j[s, c]      (PSUM accumulate)
    outT = (W^T @ aggT) * dis[col] + bias[d_out]        (on-device tail)

Device mapping (destination-sharded, 8 cores, SPMD):
  - 80 node tiles of 128; core j owns 10 consecutive dest tiles (1280 cols).
  - A blocks are {0,1,2,..} edge counts, EXACT in fp8e4 -> rhs stream is
    12.6 MB/core of sequential HBM reads (no gather, no SWDGE descriptors).
  - g is f16 [128, 80*128] (partition-major, host-prepped); lhsT = g_j.
  - 80 x 3 accumulating matmuls into three PSUM regions [d, 512|512|256].
  - tail: PSUM -> f16, 3 W-matmuls (f16), scale by dis[col] (broadcast
    tile), + bias (per-partition), write outT [128, 1280]; host transposes.
"""

import numpy as np

import concourse.bacc as bacc
import concourse.bass as bass
import concourse.mybir as mybir
import concourse.tile as tile
from concourse import bass_utils

N_NODES = 10000
N_EDGES = 640000
D = 128
P = 128
NCORES = 8
NT = 80                  # node tiles (src and dest)
NPAD = NT * P            # 10240
NTS = 79                 # src tiles actually processed (tile 79 is all-pad)
TPC = NT // NCORES       # 10 dest tiles per core
CPC = TPC * P            # 1280 dest columns per core
CHUNKS = (8, 8, 8, 8, 8, 8, 8, 8, 8, 7)   # src-tile chunking (sums to NTS)
CG = (512, 512, 256)     # dest column groups per matmul (PSUM bank limit)

f32 = mybir.dt.float32
f16 = mybir.dt.float16
f8 = mybir.dt.float8e4


def _build_inputs(x, edge_index, W, bias):
    """Host-side prep: prescaled g table (f16, partition-major), per-core
    fp8 adjacency-count blocks, per-core dest scales."""
    row = edge_index[0].astype(np.int64)
    col = edge_index[1].astype(np.int64)

    deg = np.bincount(row, minlength=N_NODES).astype(np.float64) + 1.0
    dis = (deg ** -0.5).astype(np.float32)
    dis_pad = np.zeros(NPAD, np.float32)
    dis_pad[:N_NODES] = dis

    g_pad = np.zeros((NPAD, D), np.float32)
    g_pad[:N_NODES] = x * dis[:, None]
    g_sb = np.ascontiguousarray(
        g_pad.reshape(NT, P, D).transpose(1, 0, 2).reshape(P, NT * D)
    ).astype(np.float16)

    f8np = mybir.dt.np(f8)
    W16 = W.astype(np.float16)
    bias_p = np.ascontiguousarray(bias.reshape(D, 1)).astype(np.float32)

    in_maps = []
    for j in range(NCORES):
        lo, hi = j * CPC, (j + 1) * CPC
        m = (col >= lo) & (col < hi)
        r = row[m]
        c = col[m] - lo
        sl = np.arange(lo, min(hi, N_NODES), dtype=np.int64)
        rr = np.concatenate([r, sl])
        cc = np.concatenate([c, sl - lo])
        cnt = np.bincount(rr * CPC + cc, minlength=NPAD * CPC)
        mx = cnt.max()
        assert mx <= 8, f"edge multiplicity {mx} not exact in fp8e4"
        A = np.ascontiguousarray(
            cnt.reshape(NT, P, CPC).transpose(1, 0, 2)[:, :NTS]
            .reshape(P, NTS * CPC)
        ).astype(np.float32).astype(f8np)
        in_maps.append(
            {
                "g_sb": g_sb,
                "A": A,
                "W16": W16,
                "diss": dis_pad[lo:hi].reshape(1, CPC).copy(),
                "bias_p": bias_p,
            }
        )
    return in_maps


def _build_program(loop_n=1):
    nc = bacc.Bacc("TRN2", target_bir_lowering=False, debug=False,
                   num_devices=NCORES)
    g_d = nc.dram_tensor("g_sb", [P, NT * D], f16, kind="ExternalInput")
    a_d = nc.dram_tensor("A", [P, NTS * CPC], f8, kind="ExternalInput")
    w_d = nc.dram_tensor("W16", [D, D], f16, kind="ExternalInput")
    diss_d = nc.dram_tensor("diss", [1, CPC], f32, kind="ExternalInput")
    bias_d = nc.dram_tensor("bias_p", [D, 1], f32, kind="ExternalInput")
    out_d = nc.dram_tensor("outT", [P, CPC], f32, kind="ExternalOutput")

    with tile.TileContext(nc) as tc:
        with (
            tc.tile_pool(name="const", bufs=1) as cpool,
            tc.tile_pool(name="astr", bufs=10) as apool,
            tc.tile_pool(name="tail", bufs=2) as spool,
            tc.tile_pool(name="pacc", bufs=1, space="PSUM") as pgpool,
            tc.tile_pool(name="pout", bufs=1, space="PSUM") as popool,
        ):

            def _consts():
                g_t = cpool.tile([P, NT * D], f16)
                nc.gpsimd.dma_start(out=g_t[:], in_=g_d.ap())
                w_t = cpool.tile([D, D], f16)
                nc.sync.dma_start(out=w_t[:], in_=w_d.ap())
                diss_b = cpool.tile([P, CPC], f32)
                nc.gpsimd.dma_start(
                    out=diss_b[:], in_=diss_d.ap()[0].partition_broadcast(P)
                )
                bias_t = cpool.tile([P, 1], f32)
                nc.sync.dma_start(out=bias_t[:], in_=bias_d.ap())
                return g_t, w_t, diss_b, bias_t

            def _body(g_t, w_t, diss_b, bias_t):
                pg = [pgpool.tile([P, n], f32, tag=f"pg{k}", name=f"pg{k}")
                      for k, n in enumerate(CG)]
                j = 0
                for jc, ch in enumerate(CHUNKS):
                    a_t = apool.tile([P, ch * CPC], f8, tag="a",
                                     name=f"a{jc}")
                    eng = nc.sync if jc % 2 == 0 else nc.scalar
                    eng.dma_start(
                        out=a_t[:],
                        in_=a_d.ap()[:, j * CPC:(j + ch) * CPC],
                    )
                    for jl in range(ch):
                        lhs = g_t[:, j * D:(j + 1) * D]
                        base = jl * CPC
                        off = 0
                        for k, n in enumerate(CG):
                            nc.tensor.matmul(
                                pg[k][:],
                                lhsT=lhs,
                                rhs=a_t[:, base + off:base + off + n],
                                start=(j == 0),
                                stop=(j == NTS - 1),
                            )
                            off += n
                        j += 1

                # evacuate PSUM with the dis[col] scale fused in (f16 out)
                aggT = spool.tile([P, CPC], f16, tag="aggT")
                off = 0
                for k, n in enumerate(CG):
                    nc.vector.tensor_mul(out=aggT[:, off:off + n],
                                         in0=pg[k][:],
                                         in1=diss_b[:, off:off + n])
                    off += n
                po = [popool.tile([P, n], f32, tag=f"po{k}", name=f"po{k}")
                      for k, n in enumerate(CG)]
                off = 0
                for k, n in enumerate(CG):
                    nc.tensor.matmul(po[k][:], lhsT=w_t[:],
                                     rhs=aggT[:, off:off + n],
                                     start=True, stop=True)
                    off += n
                o_t = spool.tile([P, CPC], f32, tag="o")
                off = 0
                for k, n in enumerate(CG):
                    nc.vector.tensor_scalar_add(o_t[:, off:off + n],
                                                po[k][:], bias_t[:, 0:1])
                    off += n
                nc.gpsimd.dma_start(out=out_d.ap(), in_=o_t[:])

            consts = _consts()
            if loop_n == 1:
                _body(*consts)
            else:
                with tc.For_i(0, loop_n, 1):
                    _body(*consts)

    nc.compile()
    return nc


def kernel(x, edge_index, W, bias):
    x = np.asarray(x, dtype=np.float32)
    edge_index = np.asarray(edge_index)
    W = np.asarray(W, dtype=np.float32)
    bias = np.asarray(bias, dtype=np.float32)
    assert x.shape == (N_NODES, D) and edge_index.shape == (2, N_EDGES)

    in_maps = _build_inputs(x, edge_index, W, bias)
    nc = _build_program()
    res = bass_utils.run_bass_kernel_spmd(nc, in_maps,
                                          core_ids=list(range(NCORES)))

    out = np.empty((NCORES * CPC, D), np.float32)
    for j in range(NCORES):
        out[j * CPC:(j + 1) * CPC] = res.results[j]["outT"].T
    return out[:N_NODES]


# revision 13
# speedup vs baseline: 1.4413x; 1.4413x over previous
"""GCNConv (N=10000, E=640000, D=128) on 8 Trainium2 NeuronCores.

Math: out = diag(dis) (A + I) diag(dis) x W + bias, dis = deg^-1/2 with deg
over edge_index[0] (+1 self-loop).  Since the edge weight factorizes as
dis[row]*dis[col], fold dis[row] into a host-prescaled table
g = diag(dis) x  and dis[col] into a post-scale.  The aggregation then
becomes a sum of DENSE block matmuls against an integer-count adjacency:

    aggT[d, c] = sum_j  g_j[s, d]^T  @  A_j[s, c]      (PSUM accumulate)
    outT = (W^T @ aggT) * dis[col] + bias[d_out]        (on-device tail)

Device mapping (destination-sharded, 8 cores, SPMD):
  - 80 node tiles of 128; core j owns 10 consecutive dest tiles (1280 cols).
  - A blocks are {0,1,2,..} edge counts, EXACT in fp8e4 -> rhs stream is
    12.6 MB/core of sequential HBM reads (no gather, no SWDGE descriptors).
  - g is f16 [128, 80*128] (partition-major, host-prepped); lhsT = g_j.
  - 80 x 3 accumulating matmuls into three PSUM regions [d, 512|512|256].
  - tail: PSUM -> f16, 3 W-matmuls (f16), scale by dis[col] (broadcast
    tile), + bias (per-partition), write outT [128, 1280]; host transposes.
"""

import numpy as np

import concourse.bacc as bacc
import concourse.bass as bass
import concourse.mybir as mybir
import concourse.tile as tile
from concourse import bass_utils

N_NODES = 10000
N_EDGES = 640000
D = 128
P = 128
NCORES = 8
NT = 80                  # node tiles (src and dest)
NPAD = NT * P            # 10240
NTS = 79                 # src tiles actually processed (tile 79 is all-pad)
TPC = NT // NCORES       # 10 dest tiles per core
CPC = TPC * P            # 1280 dest columns per core
CHUNKS = (8, 8, 8, 8, 8, 8, 8, 8, 8, 7)   # src-tile chunking (sums to NTS)
CG = (512, 512, 256)     # dest column groups per matmul (PSUM bank limit)

f32 = mybir.dt.float32
f16 = mybir.dt.float16
f8 = mybir.dt.float8e4


def _build_inputs(x, edge_index, W, bias):
    """Host-side prep: prescaled g table (f16, partition-major), per-core
    fp8 adjacency-count blocks, per-core dest scales."""
    row = edge_index[0].astype(np.int64)
    col = edge_index[1].astype(np.int64)

    deg = np.bincount(row, minlength=N_NODES).astype(np.float64) + 1.0
    dis = (deg ** -0.5).astype(np.float32)
    dis_pad = np.zeros(NPAD, np.float32)
    dis_pad[:N_NODES] = dis

    g_pad = np.zeros((NPAD, D), np.float32)
    g_pad[:N_NODES] = x * dis[:, None]
    g_sb = np.ascontiguousarray(
        g_pad.reshape(NT, P, D).transpose(1, 0, 2).reshape(P, NT * D)
    ).astype(np.float16)

    f8np = mybir.dt.np(f8)
    W16 = W.astype(np.float16)
    bias_p = np.ascontiguousarray(bias.reshape(D, 1)).astype(np.float32)

    in_maps = []
    for j in range(NCORES):
        lo, hi = j * CPC, (j + 1) * CPC
        m = (col >= lo) & (col < hi)
        r = row[m]
        c = col[m] - lo
        sl = np.arange(lo, min(hi, N_NODES), dtype=np.int64)
        rr = np.concatenate([r, sl])
        cc = np.concatenate([c, sl - lo])
        cnt = np.bincount(rr * CPC + cc, minlength=NPAD * CPC)
        mx = cnt.max()
        assert mx <= 8, f"edge multiplicity {mx} not exact in fp8e4"
        A = np.ascontiguousarray(
            cnt.reshape(NT, P, CPC).transpose(1, 0, 2)[:, :NTS]
            .reshape(P, NTS * CPC)
        ).astype(np.float32).astype(f8np)
        in_maps.append(
            {
                "g_sb": g_sb,
                "A": A,
                "W16": W16,
                "diss": dis_pad[lo:hi].reshape(1, CPC).copy(),
                "bias_p": bias_p,
            }
        )
    return in_maps


def _build_program(loop_n=1):
    nc = bacc.Bacc("TRN2", target_bir_lowering=False, debug=False,
                   num_devices=NCORES)
    g_d = nc.dram_tensor("g_sb", [P, NT * D], f16, kind="ExternalInput")
    a_d = nc.dram_tensor("A", [P, NTS * CPC], f8, kind="ExternalInput")
    w_d = nc.dram_tensor("W16", [D, D], f16, kind="ExternalInput")
    diss_d = nc.dram_tensor("diss", [1, CPC], f32, kind="ExternalInput")
    bias_d = nc.dram_tensor("bias_p", [D, 1], f32, kind="ExternalInput")
    out_d = nc.dram_tensor("outT", [P, CPC], f32, kind="ExternalOutput")

    with tile.TileContext(nc) as tc:
        with (
            tc.tile_pool(name="const", bufs=1) as cpool,
            tc.tile_pool(name="astr", bufs=10) as apool,
            tc.tile_pool(name="tail", bufs=2) as spool,
            tc.tile_pool(name="pacc", bufs=2, space="PSUM") as pgpool,
            tc.tile_pool(name="pout", bufs=1, space="PSUM") as popool,
        ):

            def _consts():
                g_t = cpool.tile([P, NT * D], f16)
                nc.gpsimd.dma_start(out=g_t[:], in_=g_d.ap())
                w_t = cpool.tile([D, D], f16)
                nc.sync.dma_start(out=w_t[:], in_=w_d.ap())
                diss_b = cpool.tile([P, CPC], f32)
                nc.gpsimd.dma_start(
                    out=diss_b[:], in_=diss_d.ap()[0].partition_broadcast(P)
                )
                bias_t = cpool.tile([P, 1], f32)
                nc.sync.dma_start(out=bias_t[:], in_=bias_d.ap())
                return g_t, w_t, diss_b, bias_t

            def _body(g_t, w_t, diss_b, bias_t):
                pg = [
                    pgpool.tile([P, CG[0]], f32, tag="pg0", name="pg0"),
                    pgpool.tile([P, CG[1]], f32, tag="pg1", name="pg1"),
                    popool.tile([P, CG[2]], f32, tag="pg2", name="pg2"),
                ]
                j = 0
                for jc, ch in enumerate(CHUNKS):
                    a_t = apool.tile([P, ch * CPC], f8, tag="a",
                                     name=f"a{jc}")
                    eng = nc.sync if jc % 2 == 0 else nc.scalar
                    eng.dma_start(
                        out=a_t[:],
                        in_=a_d.ap()[:, j * CPC:(j + ch) * CPC],
                    )
                    for jl in range(ch):
                        lhs = g_t[:, j * D:(j + 1) * D]
                        base = jl * CPC
                        off = 0
                        for k, n in enumerate(CG):
                            nc.tensor.matmul(
                                pg[k][:],
                                lhsT=lhs,
                                rhs=a_t[:, base + off:base + off + n],
                                start=(j == 0),
                                stop=(j == NTS - 1),
                            )
                            off += n
                        j += 1

                # evacuate PSUM with the dis[col] scale fused in (f16 out)
                aggT = spool.tile([P, CPC], f16, tag="aggT")
                off = 0
                for k, n in enumerate(CG):
                    nc.vector.tensor_mul(out=aggT[:, off:off + n],
                                         in0=pg[k][:],
                                         in1=diss_b[:, off:off + n])
                    off += n
                po = [popool.tile([P, n], f32, tag=f"po{k}", name=f"po{k}")
                      for k, n in enumerate(CG)]
                off = 0
                for k, n in enumerate(CG):
                    nc.tensor.matmul(po[k][:], lhsT=w_t[:],
                                     rhs=aggT[:, off:off + n],
                                     start=True, stop=True)
                    off += n
                o_t = spool.tile([P, CPC], f32, tag="o")
                off = 0
                for k, n in enumerate(CG):
                    nc.vector.tensor_scalar_add(o_t[:, off:off + n],
                                                po[k][:], bias_t[:, 0:1])
                    off += n
                nc.gpsimd.dma_start(out=out_d.ap(), in_=o_t[:])

            consts = _consts()
            for _ in range(loop_n):
                _body(*consts)

    nc.compile()
    return nc


def kernel(x, edge_index, W, bias):
    x = np.asarray(x, dtype=np.float32)
    edge_index = np.asarray(edge_index)
    W = np.asarray(W, dtype=np.float32)
    bias = np.asarray(bias, dtype=np.float32)
    assert x.shape == (N_NODES, D) and edge_index.shape == (2, N_EDGES)

    in_maps = _build_inputs(x, edge_index, W, bias)
    nc = _build_program()
    res = bass_utils.run_bass_kernel_spmd(nc, in_maps,
                                          core_ids=list(range(NCORES)))

    out = np.empty((NCORES * CPC, D), np.float32)
    for j in range(NCORES):
        out[j * CPC:(j + 1) * CPC] = res.results[j]["outT"].T
    return out[:N_NODES]


# revision 14
# speedup vs baseline: 1.4451x; 1.0027x over previous
"""GCNConv (N=10000, E=640000, D=128) on 8 Trainium2 NeuronCores.

Math: out = diag(dis) (A + I) diag(dis) x W + bias, dis = deg^-1/2 with deg
over edge_index[0] (+1 self-loop).  Since the edge weight factorizes as
dis[row]*dis[col], fold dis[row] into a host-prescaled table
g = diag(dis) x  and dis[col] into a post-scale.  The aggregation then
becomes a sum of DENSE block matmuls against an integer-count adjacency:

    aggT[d, c] = sum_j  g_j[s, d]^T  @  A_j[s, c]      (PSUM accumulate)
    outT = (W^T @ aggT) * dis[col] + bias[d_out]        (on-device tail)

Device mapping (destination-sharded, 8 cores, SPMD):
  - 80 node tiles of 128; core j owns 10 consecutive dest tiles (1280 cols).
  - A blocks are {0,1,2,..} edge counts, EXACT in fp8e4 -> rhs stream is
    12.6 MB/core of sequential HBM reads (no gather, no SWDGE descriptors).
  - g is f16 [128, 80*128] (partition-major, host-prepped); lhsT = g_j.
  - 80 x 3 accumulating matmuls into three PSUM regions [d, 512|512|256].
  - tail: PSUM -> f16, 3 W-matmuls (f16), scale by dis[col] (broadcast
    tile), + bias (per-partition), write outT [128, 1280]; host transposes.
"""

import numpy as np

import concourse.bacc as bacc
import concourse.bass as bass
import concourse.mybir as mybir
import concourse.tile as tile
from concourse import bass_utils

N_NODES = 10000
N_EDGES = 640000
D = 128
P = 128
NCORES = 8
NT = 80                  # node tiles (src and dest)
NPAD = NT * P            # 10240
NTS = 79                 # src tiles actually processed (tile 79 is all-pad)
CPC = N_NODES // NCORES  # 1250 dest columns per core (exact, no pad dests)
CHUNKS = (2, 6, 8, 8, 8, 8, 8, 8, 8, 8, 7)  # src-tile chunking (sums to NTS)
CG = (512, 512, 226)     # dest column groups per matmul (PSUM bank limit)

f32 = mybir.dt.float32
f16 = mybir.dt.float16
f8 = mybir.dt.float8e4


def _build_inputs(x, edge_index, W, bias):
    """Host-side prep: prescaled g table (f16, partition-major), per-core
    fp8 adjacency-count blocks, per-core dest scales."""
    row = edge_index[0].astype(np.int64)
    col = edge_index[1].astype(np.int64)

    deg = np.bincount(row, minlength=N_NODES).astype(np.float64) + 1.0
    dis = (deg ** -0.5).astype(np.float32)
    dis_pad = np.zeros(NPAD, np.float32)
    dis_pad[:N_NODES] = dis

    g_pad = np.zeros((NPAD, D), np.float32)
    g_pad[:N_NODES] = x * dis[:, None]
    g_sb = np.ascontiguousarray(
        g_pad.reshape(NT, P, D).transpose(1, 0, 2).reshape(P, NT * D)
    ).astype(np.float16)

    f8np = mybir.dt.np(f8)
    W16 = W.astype(np.float16)
    bias_p = np.ascontiguousarray(bias.reshape(D, 1)).astype(np.float32)

    in_maps = []
    for j in range(NCORES):
        lo, hi = j * CPC, (j + 1) * CPC
        m = (col >= lo) & (col < hi)
        r = row[m]
        c = col[m] - lo
        sl = np.arange(lo, hi, dtype=np.int64)
        rr = np.concatenate([r, sl])
        cc = np.concatenate([c, sl - lo])
        cnt = np.bincount(rr * CPC + cc, minlength=NPAD * CPC)
        mx = cnt.max()
        assert mx <= 8, f"edge multiplicity {mx} not exact in fp8e4"
        A = np.ascontiguousarray(
            cnt.reshape(NT, P, CPC).transpose(1, 0, 2)[:, :NTS]
            .reshape(P, NTS * CPC)
        ).astype(np.float32).astype(f8np)
        in_maps.append(
            {
                "g_sb": g_sb,
                "A": A,
                "W16": W16,
                "diss": dis_pad[lo:hi].reshape(1, CPC).copy(),
                "bias_p": bias_p,
            }
        )
    return in_maps


def _build_program(loop_n=1):
    nc = bacc.Bacc("TRN2", target_bir_lowering=False, debug=False,
                   num_devices=NCORES)
    g_d = nc.dram_tensor("g_sb", [P, NT * D], f16, kind="ExternalInput")
    a_d = nc.dram_tensor("A", [P, NTS * CPC], f8, kind="ExternalInput")
    w_d = nc.dram_tensor("W16", [D, D], f16, kind="ExternalInput")
    diss_d = nc.dram_tensor("diss", [1, CPC], f32, kind="ExternalInput")
    bias_d = nc.dram_tensor("bias_p", [D, 1], f32, kind="ExternalInput")
    out_d = nc.dram_tensor("outT", [P, CPC], f32, kind="ExternalOutput")

    with tile.TileContext(nc) as tc:
        with (
            tc.tile_pool(name="const", bufs=1) as cpool,
            tc.tile_pool(name="astr", bufs=10) as apool,
            tc.tile_pool(name="tail", bufs=2) as spool,
            tc.tile_pool(name="pacc", bufs=2, space="PSUM") as pgpool,
            tc.tile_pool(name="pout", bufs=1, space="PSUM") as popool,
        ):

            def _consts():
                g_t = cpool.tile([P, NT * D], f16)
                nc.gpsimd.dma_start(out=g_t[:, :16 * D],
                                    in_=g_d.ap()[:, :16 * D])
                nc.gpsimd.dma_start(out=g_t[:, 16 * D:],
                                    in_=g_d.ap()[:, 16 * D:])
                w_t = cpool.tile([D, D], f16)
                nc.sync.dma_start(out=w_t[:], in_=w_d.ap())
                diss_b = cpool.tile([P, CPC], f32)
                nc.gpsimd.dma_start(
                    out=diss_b[:], in_=diss_d.ap()[0].partition_broadcast(P)
                )
                bias_t = cpool.tile([P, 1], f32)
                nc.sync.dma_start(out=bias_t[:], in_=bias_d.ap())
                return g_t, w_t, diss_b, bias_t

            def _body(g_t, w_t, diss_b, bias_t):
                pg = [
                    pgpool.tile([P, CG[0]], f32, tag="pg0", name="pg0"),
                    pgpool.tile([P, CG[1]], f32, tag="pg1", name="pg1"),
                    popool.tile([P, CG[2]], f32, tag="pg2", name="pg2"),
                ]
                j = 0
                for jc, ch in enumerate(CHUNKS):
                    a_t = apool.tile([P, ch * CPC], f8, tag="a",
                                     name=f"a{jc}")
                    eng = nc.sync if jc % 2 == 0 else nc.scalar
                    eng.dma_start(
                        out=a_t[:],
                        in_=a_d.ap()[:, j * CPC:(j + ch) * CPC],
                    )
                    for jl in range(ch):
                        lhs = g_t[:, j * D:(j + 1) * D]
                        base = jl * CPC
                        off = 0
                        for k, n in enumerate(CG):
                            nc.tensor.matmul(
                                pg[k][:],
                                lhsT=lhs,
                                rhs=a_t[:, base + off:base + off + n],
                                start=(j == 0),
                                stop=(j == NTS - 1),
                            )
                            off += n
                        j += 1

                # evacuate PSUM with the dis[col] scale fused in (f16 out)
                aggT = spool.tile([P, CPC], f16, tag="aggT")
                off = 0
                for k, n in enumerate(CG):
                    nc.vector.tensor_mul(out=aggT[:, off:off + n],
                                         in0=pg[k][:],
                                         in1=diss_b[:, off:off + n])
                    off += n
                po = [popool.tile([P, n], f32, tag=f"po{k}", name=f"po{k}")
                      for k, n in enumerate(CG)]
                off = 0
                for k, n in enumerate(CG):
                    nc.tensor.matmul(po[k][:], lhsT=w_t[:],
                                     rhs=aggT[:, off:off + n],
                                     start=True, stop=True)
                    off += n
                o_t = spool.tile([P, CPC], f32, tag="o")
                off = 0
                for k, n in enumerate(CG):
                    nc.vector.tensor_scalar_add(o_t[:, off:off + n],
                                                po[k][:], bias_t[:, 0:1])
                    off += n
                nc.gpsimd.dma_start(out=out_d.ap(), in_=o_t[:])

            consts = _consts()
            for _ in range(loop_n):
                _body(*consts)

    nc.compile()
    return nc


def kernel(x, edge_index, W, bias):
    x = np.asarray(x, dtype=np.float32)
    edge_index = np.asarray(edge_index)
    W = np.asarray(W, dtype=np.float32)
    bias = np.asarray(bias, dtype=np.float32)
    assert x.shape == (N_NODES, D) and edge_index.shape == (2, N_EDGES)

    in_maps = _build_inputs(x, edge_index, W, bias)
    nc = _build_program()
    res = bass_utils.run_bass_kernel_spmd(nc, in_maps,
                                          core_ids=list(range(NCORES)))

    out = np.empty((N_NODES, D), np.float32)
    for j in range(NCORES):
        out[j * CPC:(j + 1) * CPC] = res.results[j]["outT"].T
    return out


# revision 15
# speedup vs baseline: 1.4607x; 1.0108x over previous
"""GCNConv (N=10000, E=640000, D=128) on 8 Trainium2 NeuronCores.

Math: out = diag(dis) (A + I) diag(dis) x W + bias, dis = deg^-1/2 with deg
over edge_index[0] (+1 self-loop).  Since the edge weight factorizes as
dis[row]*dis[col], fold dis[row] into a host-prescaled table
g = diag(dis) x  and dis[col] into a post-scale.  The aggregation then
becomes a sum of DENSE block matmuls against an integer-count adjacency:

    aggT[d, c] = sum_j  g_j[s, d]^T  @  A_j[s, c]      (PSUM accumulate)
    outT = (W^T @ aggT) * dis[col] + bias[d_out]        (on-device tail)

Device mapping (destination-sharded, 8 cores, SPMD):
  - 80 node tiles of 128; core j owns 10 consecutive dest tiles (1280 cols).
  - A blocks are {0,1,2,..} edge counts, EXACT in fp8e4 -> rhs stream is
    12.6 MB/core of sequential HBM reads (no gather, no SWDGE descriptors).
  - g is f16 [128, 80*128] (partition-major, host-prepped); lhsT = g_j.
  - 80 x 3 accumulating matmuls into three PSUM regions [d, 512|512|256].
  - tail: PSUM -> f16, 3 W-matmuls (f16), scale by dis[col] (broadcast
    tile), + bias (per-partition), write outT [128, 1280]; host transposes.
"""

import numpy as np

import concourse.bacc as bacc
import concourse.bass as bass
import concourse.mybir as mybir
import concourse.tile as tile
from concourse import bass_utils

N_NODES = 10000
N_EDGES = 640000
D = 128
P = 128
NCORES = 8
NT = 80                  # node tiles (src and dest)
NPAD = NT * P            # 10240
NTS = 79                 # src tiles actually processed (tile 79 is all-pad)
CPC = N_NODES // NCORES  # 1250 dest columns per core (exact, no pad dests)
CHUNKS = (2, 6, 8, 8, 8, 8, 8, 8, 8, 8, 7)  # src-tile chunking (sums to NTS)
CG = (512, 512, 226)     # dest column groups per matmul (PSUM bank limit)

f32 = mybir.dt.float32
f16 = mybir.dt.float16
f8 = mybir.dt.float8e4


def _build_inputs(x, edge_index, W, bias):
    """Host-side prep: prescaled g table (f16, partition-major), per-core
    fp8 adjacency-count blocks, per-core dest scales."""
    row = edge_index[0].astype(np.int64)
    col = edge_index[1].astype(np.int64)

    deg = np.bincount(row, minlength=N_NODES).astype(np.float64) + 1.0
    dis = (deg ** -0.5).astype(np.float32)
    dis_pad = np.zeros(NPAD, np.float32)
    dis_pad[:N_NODES] = dis

    g_pad = np.zeros((NPAD, D), np.float32)
    g_pad[:N_NODES] = x * dis[:, None]
    g_sb = np.ascontiguousarray(
        g_pad.reshape(NT, P, D).transpose(1, 0, 2).reshape(P, NT * D)
    ).astype(np.float16)

    f8np = mybir.dt.np(f8)
    W16 = W.astype(np.float16)
    bias_p = np.ascontiguousarray(bias.reshape(D, 1)).astype(np.float32)

    in_maps = []
    for j in range(NCORES):
        lo, hi = j * CPC, (j + 1) * CPC
        m = (col >= lo) & (col < hi)
        r = row[m]
        c = col[m] - lo
        sl = np.arange(lo, hi, dtype=np.int64)
        rr = np.concatenate([r, sl])
        cc = np.concatenate([c, sl - lo])
        cnt = np.bincount(rr * CPC + cc, minlength=NPAD * CPC)
        mx = cnt.max()
        assert mx <= 8, f"edge multiplicity {mx} not exact in fp8e4"
        A = np.ascontiguousarray(
            cnt.reshape(NT, P, CPC).transpose(1, 0, 2)[:, :NTS]
            .reshape(P, NTS * CPC)
        ).astype(np.float32).astype(f8np)
        in_maps.append(
            {
                "g_sb": g_sb,
                "A": A,
                "W16": W16,
                "diss": dis_pad[lo:hi].reshape(1, CPC).copy(),
                "bias_p": bias_p,
            }
        )
    return in_maps


def _build_program(loop_n=1):
    nc = bacc.Bacc("TRN2", target_bir_lowering=False, debug=False,
                   num_devices=NCORES)
    g_d = nc.dram_tensor("g_sb", [P, NT * D], f16, kind="ExternalInput")
    a_d = nc.dram_tensor("A", [P, NTS * CPC], f8, kind="ExternalInput")
    w_d = nc.dram_tensor("W16", [D, D], f16, kind="ExternalInput")
    diss_d = nc.dram_tensor("diss", [1, CPC], f32, kind="ExternalInput")
    bias_d = nc.dram_tensor("bias_p", [D, 1], f32, kind="ExternalInput")
    out_d = nc.dram_tensor("outT", [P, CPC], f32, kind="ExternalOutput")

    with tile.TileContext(nc) as tc:
        with (
            tc.tile_pool(name="const", bufs=1) as cpool,
            tc.tile_pool(name="astr", bufs=1) as apool,
            tc.tile_pool(name="tail", bufs=2) as spool,
            tc.tile_pool(name="pacc", bufs=2, space="PSUM") as pgpool,
            tc.tile_pool(name="pout", bufs=1, space="PSUM") as popool,
        ):

            def _consts():
                g_t = cpool.tile([P, NT * D], f16)
                nc.scalar.dma_start(out=g_t[:, :16 * D],
                                    in_=g_d.ap()[:, :16 * D])
                nc.gpsimd.dma_start(out=g_t[:, 16 * D:],
                                    in_=g_d.ap()[:, 16 * D:])
                w_t = cpool.tile([D, D], f16)
                nc.scalar.dma_start(out=w_t[:], in_=w_d.ap())
                diss_b = cpool.tile([P, CPC], f32)
                nc.gpsimd.dma_start(
                    out=diss_b[:], in_=diss_d.ap()[0].partition_broadcast(P)
                )
                bias_t = cpool.tile([P, 1], f32)
                nc.scalar.dma_start(out=bias_t[:], in_=bias_d.ap())
                return g_t, w_t, diss_b, bias_t

            def _body(g_t, w_t, diss_b, bias_t):
                pg = [
                    pgpool.tile([P, CG[0]], f32, tag="pg0", name="pg0"),
                    pgpool.tile([P, CG[1]], f32, tag="pg1", name="pg1"),
                    popool.tile([P, CG[2]], f32, tag="pg2", name="pg2"),
                ]
                j = 0
                for jc, ch in enumerate(CHUNKS):
                    a_t = apool.tile([P, ch * CPC], f8, tag=f"a{jc}",
                                     name=f"a{jc}")
                    nc.sync.dma_start(
                        out=a_t[:],
                        in_=a_d.ap()[:, j * CPC:(j + ch) * CPC],
                    )
                    for jl in range(ch):
                        lhs = g_t[:, j * D:(j + 1) * D]
                        base = jl * CPC
                        off = 0
                        for k, n in enumerate(CG):
                            nc.tensor.matmul(
                                pg[k][:],
                                lhsT=lhs,
                                rhs=a_t[:, base + off:base + off + n],
                                start=(j == 0),
                                stop=(j == NTS - 1),
                            )
                            off += n
                        j += 1

                # evacuate PSUM with the dis[col] scale fused in (f16 out)
                aggT = spool.tile([P, CPC], f16, tag="aggT")
                off = 0
                for k, n in enumerate(CG):
                    nc.vector.tensor_mul(out=aggT[:, off:off + n],
                                         in0=pg[k][:],
                                         in1=diss_b[:, off:off + n])
                    off += n
                po = [popool.tile([P, n], f32, tag=f"po{k}", name=f"po{k}")
                      for k, n in enumerate(CG)]
                off = 0
                for k, n in enumerate(CG):
                    nc.tensor.matmul(po[k][:], lhsT=w_t[:],
                                     rhs=aggT[:, off:off + n],
                                     start=True, stop=True)
                    off += n
                o_t = spool.tile([P, CPC], f32, tag="o")
                off = 0
                for k, n in enumerate(CG):
                    nc.vector.tensor_scalar_add(o_t[:, off:off + n],
                                                po[k][:], bias_t[:, 0:1])
                    off += n
                nc.gpsimd.dma_start(out=out_d.ap(), in_=o_t[:])

            consts = _consts()
            for _ in range(loop_n):
                _body(*consts)

    nc.compile()
    return nc


def kernel(x, edge_index, W, bias):
    x = np.asarray(x, dtype=np.float32)
    edge_index = np.asarray(edge_index)
    W = np.asarray(W, dtype=np.float32)
    bias = np.asarray(bias, dtype=np.float32)
    assert x.shape == (N_NODES, D) and edge_index.shape == (2, N_EDGES)

    in_maps = _build_inputs(x, edge_index, W, bias)
    nc = _build_program()
    res = bass_utils.run_bass_kernel_spmd(nc, in_maps,
                                          core_ids=list(range(NCORES)))

    out = np.empty((N_NODES, D), np.float32)
    for j in range(NCORES):
        out[j * CPC:(j + 1) * CPC] = res.results[j]["outT"].T
    return out
